# revision 15
# baseline (speedup 1.0000x reference)
"""Bond2AtomLayer GNN message-passing kernel for 8 TRN2 NeuronCores.

Strategy (dst-range sharding, no collectives):
  - Host sorts edges by dst and buckets them to the core that owns the dst
    node range (6250 nodes per core).  Within a core, edges are grouped
    into 128-node destination "windows"; the scatter-sum becomes one-hot
    matmuls accumulating in PSUM per window.
  - Each core computes the k-projection table for ALL nodes (src is
    unsharded) and the q-projection table for its own range, stores them
    in HBM, and per-edge k[src]/q[dst] rows are fetched with gpsimd
    dma_gather (int16 indices; k is split into src<32768 / >=32768
    regions per window since the index type is int16).
  - Edge softmax skips the segment_max pass: z is bounded (|z| < ~10)
    for this model family so exp(z) is safe in fp32, and
    ft = (sum_e bond_e * ez_e) / (sum_e ez_e) per node.
  - The dense residual MLP runs node-parallel in [feat, node] layout.

SPMD: one instruction stream for all 8 cores; the per-window tile counts
in the schedule are the max across cores (host pads with dummy edges).
"""

import math
import numpy as np
import ml_dtypes

BF16 = ml_dtypes.bfloat16

# ---------------------------------------------------------------- config ----


class Cfg:
    def __init__(self, N=50000, E=640000, HID=128, H=8, C=8, S=2, CHUNK=2048,
                 SPLIT=32768):
        self.N, self.E, self.HID, self.H, self.C, self.S = N, E, HID, H, C, S
        self.SPLIT = SPLIT  # int16 gather-index limit
        self.D = HID // H
        assert N % C == 0
        self.NPC = N // C                      # nodes per core
        self.W = math.ceil(self.NPC / 128)     # windows per core
        self.NPCpad = self.W * 128
        self.CHUNK = CHUNK                     # table-build chunk (nodes)
        self.NCH = math.ceil(N / CHUNK)        # k-table chunks
        self.NTAB = self.NCH * CHUNK           # padded k-table rows


class Sched:
    """Common (SPMD) tile schedule: windows grouped S at a time; each
    group's tile order is [lows of each window][highs of each window]."""

    def __init__(self, cfg, low_cnt, high_cnt):
        W, S = cfg.W, cfg.S
        Lw = np.ceil(low_cnt.max(axis=0) / 128.0).astype(np.int64)
        Hw = np.ceil(high_cnt.max(axis=0) / 128.0).astype(np.int64)
        empty = (Lw + Hw) == 0
        Lw[empty] = 1
        self.Lw, self.Hw = Lw, Hw
        self.ngroups = math.ceil(W / S)
        low_base = np.zeros(W, np.int64)
        high_base = np.zeros(W, np.int64)
        self.grp = []  # (t0, tmid, t1)
        t = 0
        for gi in range(self.ngroups):
            ws = range(gi * S, min(W, (gi + 1) * S))
            t0 = t
            for w in ws:
                low_base[w] = t
                t += Lw[w]
            tm = t
            for w in ws:
                high_base[w] = t
                t += Hw[w]
            self.grp.append((t0, tm, t))
        self.T = t
        self.low_base, self.high_base = low_base, high_base
        self.start_tile = np.where(Lw > 0, low_base, high_base)
        self.stop_tile = np.where(Hw > 0, high_base + Hw - 1, low_base + Lw - 1)
        win_of = np.full(self.T, -1, np.int64)
        for w in range(W):
            win_of[low_base[w] : low_base[w] + Lw[w]] = w
            win_of[high_base[w] : high_base[w] + Hw[w]] = w
        self.win_of = win_of
        self.MAXG = max(t1 - t0 for (t0, _, t1) in self.grp)


def _wrap16(arr):
    """int16 index list [L] (L % 16 == 0) -> dma_gather wrapped [128, L/16]."""
    w16 = arr.reshape(-1, 16).T  # [16, L/16]
    return np.tile(w16, (8, 1))


# ------------------------------------------------------------- host prep ----


def host_prep(cfg, inputs):
    N, E, HID, C, W = cfg.N, cfg.E, cfg.HID, cfg.C, cfg.W
    NPC, NPCpad = cfg.NPC, cfg.NPCpad

    src = np.asarray(inputs["src"], np.int32)
    dst = np.asarray(inputs["dst"], np.int32)
    bond = np.asarray(inputs["bond_embedding"], np.float32)
    atom = np.asarray(inputs["atom_embedding"], np.float32)
    dd = np.asarray(inputs["dist_decay"], np.float32).reshape(-1)

    hi = (src >= cfg.SPLIT).astype(np.int64)
    core_of = dst // NPC
    w_of = (dst % NPC) // 128
    key = ((core_of * W + w_of) * 2 + hi)
    order = np.argsort(key, kind="stable")
    s_key = key[order]
    cnt = np.bincount(s_key, minlength=C * W * 2).reshape(C, W, 2)

    sched = Sched(cfg, cnt[:, :, 0], cnt[:, :, 1])
    T = sched.T

    # per-edge slot in the packed [T*128] per-core edge list
    grp_start = np.searchsorted(s_key, np.arange(C * W * 2))
    rank = np.arange(E, dtype=np.int64) - grp_start[s_key]
    s_w = (dst[order] % NPC) // 128
    s_hi = hi[order]
    base_tile = np.where(s_hi == 0, sched.low_base[s_w], sched.high_base[s_w])
    slot = (base_tile + rank // 128) * 128 + rank % 128

    def bf(x):
        return np.ascontiguousarray(np.asarray(x, np.float32)).astype(BF16)

    atomT = np.zeros((HID, cfg.NTAB), BF16)
    atomT[:, :N] = atom.T.astype(BF16)

    shared = {
        "atomT_bf": atomT,
        "k_wT": bf(np.asarray(inputs["k_w"], np.float32).T),
        "q_wT": bf(np.asarray(inputs["q_w"], np.float32).T),
        "k_b_row": bf(np.asarray(inputs["k_b"], np.float32).reshape(1, HID)),
        "q_b_row": bf(np.asarray(inputs["q_b"], np.float32).reshape(1, HID)),
        "attn_gang": bf(
            np.tile(
                np.asarray(inputs["attn"], np.float32).reshape(1, HID),
                (128, sched.MAXG),
            )
        ),
        "iota_row": bf(np.tile(np.arange(128, dtype=np.float32), (128, 1))),
        "identity_bf": bf(np.eye(128, dtype=np.float32)),
        "ones1": bf(np.ones((1, 128), np.float32)),
    }
    for nm in ("lin1", "lin2", "r1a", "r1b", "r2a", "r2b"):
        shared[nm + "_wT"] = bf(np.asarray(inputs[nm + "_w"], np.float32).T)
        shared[nm + "_b"] = np.ascontiguousarray(
            np.asarray(inputs[nm + "_b"], np.float32).reshape(HID, 1)
        )

    s_src = src[order]
    s_dst = dst[order]
    s_dd = dd[order]

    in_maps = []
    for c in range(C):
        m = core_of[order] == c
        sl = slot[m]
        kidx = np.zeros(T * 128, np.int32)
        qidx = np.zeros(T * 128, np.int32)
        dstrel = np.full(T * 128, -1.0, np.float32)
        ddv = np.zeros(T * 128, np.float32)
        bondv = np.zeros((T * 128, HID), np.float32)
        kidx[sl] = np.where(s_src[m] >= cfg.SPLIT, s_src[m] - cfg.SPLIT, s_src[m])
        qidx[sl] = s_dst[m] - c * NPC
        dstrel[sl] = (s_dst[m] - c * NPC - s_w[m] * 128).astype(np.float32)
        ddv[sl] = s_dd[m]
        bondv[sl] = bond[order[m]]

        a0, a1 = c * NPC, min((c + 1) * NPC, N)
        atown = np.zeros((HID, NPCpad), np.float32)
        atown[:, : a1 - a0] = atom[a0:a1].T

        d = {
            "kidx": _wrap16(kidx.astype(np.int16)),
            "qidx": _wrap16(qidx.astype(np.int16)),
            # [128, T] column layouts: tile t, lane p -> [p, t]
            "dstrel": np.ascontiguousarray(dstrel.reshape(T, 128).T),
            "ddv": np.ascontiguousarray(ddv.reshape(T, 128).T),
            # [128, T, HID]: partition-contiguous group loads
            "bondv": np.ascontiguousarray(
                bondv.reshape(T, 128, HID).transpose(1, 0, 2)
            ).astype(BF16),
            "atomT_own_bf": atown.astype(BF16),
            "atomT_own_f32": atown,
        }
        d.update(shared)
        in_maps.append(d)

    return sched, in_maps


# ---------------------------------------------------------------- builder ----


def build_program(cfg, sched):
    import concourse.bass as bass
    import concourse.bacc as bacc
    import concourse.mybir as mybir
    from concourse import library_config
    from concourse.tile import TileContext

    f32 = mybir.dt.float32
    bf16 = mybir.dt.bfloat16
    i16 = mybir.dt.int16
    AF = mybir.ActivationFunctionType
    OP = mybir.AluOpType
    AX = mybir.AxisListType

    HID, H, W, S = cfg.HID, cfg.H, cfg.W, cfg.S
    SPLIT = cfg.SPLIT
    NPCpad, NTAB, NCH, CHUNK = cfg.NPCpad, cfg.NTAB, cfg.NCH, cfg.CHUNK
    T, MAXG = sched.T, sched.MAXG

    nc = bacc.Bacc("TRN2", target_bir_lowering=False)

    def inp(name, shape, dt):
        return nc.dram_tensor(name, shape, dt, kind="ExternalInput")

    atomT_bf = inp("atomT_bf", [HID, NTAB], bf16)
    atomT_own_bf = inp("atomT_own_bf", [HID, NPCpad], bf16)
    atomT_own_f32 = inp("atomT_own_f32", [HID, NPCpad], f32)
    kidx = inp("kidx", [128, T * 8], i16)
    qidx = inp("qidx", [128, T * 8], i16)
    dstrel = inp("dstrel", [128, T], f32)
    ddv = inp("ddv", [128, T], f32)
    bondv = inp("bondv", [128, T, HID], bf16)
    k_wT = inp("k_wT", [HID, HID], bf16)
    q_wT = inp("q_wT", [HID, HID], bf16)
    k_b_row = inp("k_b_row", [1, HID], bf16)
    q_b_row = inp("q_b_row", [1, HID], bf16)
    attn_gang = inp("attn_gang", [128, MAXG * HID], bf16)
    iota_row = inp("iota_row", [128, 128], bf16)
    identity_bf = inp("identity_bf", [128, 128], bf16)
    ones1 = inp("ones1", [1, 128], bf16)
    mlp_w = {}
    mlp_b = {}
    for nm in ("lin1", "lin2", "r1a", "r1b", "r2a", "r2b"):
        mlp_w[nm] = inp(nm + "_wT", [HID, HID], bf16)
        mlp_b[nm] = inp(nm + "_b", [HID, 1], f32)

    k_table = nc.dram_tensor("k_table", [NTAB, HID], bf16, kind="Internal")
    q_table = nc.dram_tensor("q_table", [NPCpad, HID], bf16, kind="Internal")
    out = nc.dram_tensor("out", [HID, NPCpad], f32, kind="ExternalOutput")

    with TileContext(nc) as tc:
        with (
            tc.tile_pool(name="const", bufs=1) as cpool,
            tc.tile_pool(name="resident", bufs=1) as rpool,
            tc.tile_pool(name="work", bufs=3) as sb,
            tc.tile_pool(name="edge", bufs=2) as eb,
            tc.tile_pool(name="ps", bufs=2, space="PSUM") as ps,
        ):
            nc.gpsimd.load_library(library_config.mlp)

            c_kwT = cpool.tile_from(k_wT[:, :])
            c_qwT = cpool.tile_from(q_wT[:, :])
            c_kb = cpool.tile_from(k_b_row[:, :])
            c_qb = cpool.tile_from(q_b_row[:, :])
            c_attn = cpool.tile_from(attn_gang[:, :])
            c_iota = cpool.tile_from(iota_row[:, :])
            c_ident = cpool.tile_from(identity_bf[:, :])
            c_ones1 = cpool.tile_from(ones1[:, :])
            c_w = {
                nm: cpool.tile_from(mlp_w[nm][:, :], name="c_w_" + nm)
                for nm in mlp_w
            }
            c_b = {
                nm: cpool.tile_from(mlp_b[nm][:, :], name="c_b_" + nm)
                for nm in mlp_b
            }
            r_kidx = rpool.tile_from(kidx[:, :])
            r_qidx = rpool.tile_from(qidx[:, :])
            r_dstrel = rpool.tile_from(dstrel[:, :])
            r_ddv = rpool.tile_from(ddv[:, :])
            r_ftT = rpool.tile([HID, NPCpad], bf16)

            # ---- phase A/B: projection tables --------------------------
            def build_table(table, srcT, rows, wT, brow):
                nchunks = math.ceil(rows / CHUNK)
                for ch in range(nchunks):
                    cs = ch * CHUNK
                    ce = min(rows, cs + CHUNK)
                    at = sb.tile([128, CHUNK], bf16, tag="atch")
                    nc.sync.dma_start(at[:, : ce - cs], srcT[:, cs:ce])
                    for ti in range(math.ceil((ce - cs) / 128)):
                        pt = ps.tile([128, HID], f32, tag="projp")
                        nc.tensor.matmul(
                            pt[:, :],
                            lhsT=at[:, ti * 128 : (ti + 1) * 128],
                            rhs=wT[:, :],
                            start=True,
                            stop=False,
                        )
                        nc.tensor.matmul(
                            pt[:, :], lhsT=c_ones1[:, :], rhs=brow[:, :],
                            start=False, stop=True,
                        )
                        st = sb.tile([128, HID], bf16, tag="kst")
                        nc.any.tensor_copy(st[:, :], pt[:, :])
                        nc.sync.dma_start(
                            table[cs + ti * 128 : cs + (ti + 1) * 128, :],
                            st[:, :],
                        )

            build_table(k_table, atomT_bf, NTAB, c_kwT, c_kb)
            build_table(q_table, atomT_own_bf, NPCpad, c_qwT, c_qb)

            # ---- phase C: edge pass ------------------------------------
            wp_of = {}
            for gi in range(sched.ngroups):
                t0, tm, t1 = sched.grp[gi]
                nt = t1 - t0
                nlow = tm - t0
                nhigh = t1 - tm
                kg = eb.tile([128, MAXG, HID], bf16, tag="kg")
                if nlow:
                    nc.gpsimd.dma_gather(
                        kg[:, 0:nlow, :],
                        k_table[0:SPLIT, :],
                        r_kidx[:, t0 * 8 : tm * 8],
                        nlow * 128,
                        nlow * 128,
                        HID,
                        single_packet=False,
                    )
                if nhigh:
                    nc.gpsimd.dma_gather(
                        kg[:, nlow:nt, :],
                        k_table[SPLIT:NTAB, :],
                        r_kidx[:, tm * 8 : t1 * 8],
                        nhigh * 128,
                        nhigh * 128,
                        HID,
                        single_packet=False,
                    )
                qg = eb.tile([128, MAXG, HID], bf16, tag="qg")
                nc.gpsimd.dma_gather(
                    qg[:, 0:nt, :],
                    q_table[:, :],
                    r_qidx[:, t0 * 8 : t1 * 8],
                    nt * 128,
                    nt * 128,
                    HID,
                    single_packet=False,
                )
                bo = eb.tile([128, MAXG, HID], bf16, tag="bo")
                nc.sync.dma_start(bo[:, 0:nt, :], bondv[:, t0:t1, :])

                nf = nt * HID
                esum = eb.tile([128, MAXG * HID], bf16, tag="ew")
                nc.vector.tensor_tensor(
                    esum[:, :nf],
                    kg[:, 0:nt, :].rearrange("p t f -> p (t f)"),
                    qg[:, 0:nt, :].rearrange("p t f -> p (t f)"),
                    op=OP.add,
                )
                eact = eb.tile([128, MAXG * HID], bf16, tag="eact")
                nc.scalar.activation(
                    eact[:, :nf], esum[:, :nf], AF.Lrelu, alpha=0.01
                )
                ew = eb.tile([128, MAXG * HID], bf16, tag="ew")
                nc.vector.tensor_tensor(
                    ew[:, :nf], eact[:, :nf], c_attn[:, :nf], op=OP.mult
                )
                att = eb.tile([128, MAXG, H], f32, tag="att")
                nc.vector.reduce_sum(
                    att[:, 0:nt, :],
                    ew[:, :nf].rearrange("p (a d) -> p a d", d=cfg.D),
                    axis=AX.X,
                )
                z = eb.tile([128, MAXG, H], f32, tag="z")
                nc.vector.tensor_tensor(
                    z[:, 0:nt, :],
                    att[:, 0:nt, :],
                    r_ddv[:, t0:t1].broadcast_to([128, nt, H]),
                    op=OP.add,
                )
                ez = eb.tile([128, MAXG, H], bf16, tag="ez")
                nc.scalar.activation(ez[:, 0:nt, :], z[:, 0:nt, :], AF.Exp)
                # m = bond * ez (broadcast over D), in place on bo (gpsimd)
                nc.gpsimd.tensor_tensor(
                    bo[:, 0:nt, :].rearrange("p t (h d) -> p t h d", h=H),
                    bo[:, 0:nt, :].rearrange("p t (h d) -> p t h d", h=H),
                    ez[:, 0:nt, :].broadcast_to([128, nt, H, cfg.D]),
                    op=OP.mult,
                )
                for b in range(nt):
                    t = t0 + b
                    w = int(sched.win_of[t])
                    first = t == int(sched.start_tile[w])
                    last = t == int(sched.stop_tile[w])
                    oh = eb.tile([128, 128], bf16, tag="oh", bufs=6)
                    nc.vector.tensor_scalar(
                        oh[:, :],
                        c_iota[:, :],
                        r_dstrel[:, t : t + 1],
                        None,
                        op0=OP.is_equal,
                    )
                    if first:
                        wp_of[w] = ps.tile(
                            [128, HID + H], f32, tag="wp", bufs=4, name="wp"
                        )
                    wp = wp_of[w]
                    # one start/stop per PSUM bank: start=True lazily zeroes
                    # the whole 2KB zero-region, so only the first (num)
                    # matmul starts and only the final (den) matmul stops.
                    nc.tensor.matmul(
                        wp[:, 0:HID], lhsT=oh[:, :], rhs=bo[:, b, :],
                        start=first, stop=False,
                    )
                    nc.tensor.matmul(
                        wp[:, HID : HID + H], lhsT=oh[:, :], rhs=ez[:, b, :],
                        start=False, stop=last,
                    )
                    if last:
                        den = sb.tile([128, H], f32, tag="den")
                        nc.vector.tensor_scalar(
                            den[:, :], wp[:, HID : HID + H], 1e-6, None,
                            op0=OP.add,
                        )
                        recip = sb.tile([128, H], f32, tag="recip")
                        nc.vector.reciprocal(recip[:, :], den[:, :])
                        ftw = sb.tile([128, HID], bf16, tag="ftw")
                        nc.vector.tensor_tensor(
                            ftw[:, :].rearrange("p (h d) -> p h d", h=H),
                            wp[:, 0:HID].rearrange("p (h d) -> p h d", h=H),
                            recip[:, :].broadcast_to([128, H, cfg.D]),
                            op=OP.mult,
                        )
                        ftp = ps.tile([128, 128], bf16, tag="ftp", bufs=1)
                        nc.tensor.transpose(
                            ftp[:, :], ftw[:, :], identity=c_ident[:, :]
                        )
                        nc.any.tensor_copy(
                            r_ftT[:, w * 128 : (w + 1) * 128], ftp[:, :]
                        )
                        del wp_of[w]

            # ---- phase D: dense output path + residual MLPs ------------
            NCHK = 512
            nchunks = math.ceil(NPCpad / NCHK)
            for ci in range(nchunks):
                s = ci * NCHK
                e = min(NPCpad, s + NCHK)
                n = e - s
                p1 = ps.tile([128, NCHK], f32, tag="projp")
                nc.tensor.matmul(
                    p1[:, :n], lhsT=c_w["lin1"][:, :], rhs=r_ftT[:, s:e],
                    start=True, stop=True,
                )
                t1 = sb.tile([128, NCHK], bf16, tag="t1")
                nc.scalar.activation(
                    t1[:, :n], p1[:, :n], AF.Relu, bias=c_b["lin1"][:, :]
                )
                p2 = ps.tile([128, NCHK], f32, tag="projq", bufs=1)
                nc.tensor.matmul(
                    p2[:, :n], lhsT=c_w["lin2"][:, :], rhs=t1[:, :n],
                    start=True, stop=True,
                )
                h2 = sb.tile([128, NCHK], f32, tag="h2")
                nc.scalar.activation(
                    h2[:, :n], p2[:, :n], AF.Identity, bias=c_b["lin2"][:, :]
                )
                atf = sb.tile([128, NCHK], f32, tag="atf")
                nc.sync.dma_start(atf[:, :n], atomT_own_f32[:, s:e])
                he = sb.tile([128, NCHK], f32, tag="he")
                nc.vector.tensor_tensor(
                    he[:, :n], h2[:, :n], atf[:, :n], op=OP.add
                )
                hebf = sb.tile([128, NCHK], bf16, tag="hebf")
                nc.vector.tensor_copy(hebf[:, :n], he[:, :n])

                for blk in ("r1", "r2"):
                    pa = ps.tile([128, NCHK], f32, tag="projp")
                    nc.tensor.matmul(
                        pa[:, :n], lhsT=c_w[blk + "a"][:, :], rhs=hebf[:, :n],
                        start=True, stop=True,
                    )
                    ta = sb.tile([128, NCHK], bf16, tag="t1")
                    nc.scalar.activation(
                        ta[:, :n], pa[:, :n], AF.Relu, bias=c_b[blk + "a"][:, :]
                    )
                    pb = ps.tile([128, NCHK], f32, tag="projq", bufs=1)
                    nc.tensor.matmul(
                        pb[:, :n], lhsT=c_w[blk + "b"][:, :], rhs=ta[:, :n],
                        start=True, stop=True,
                    )
                    tb = sb.tile([128, NCHK], bf16, tag="t2")
                    nc.scalar.activation(
                        tb[:, :n], pb[:, :n], AF.Relu, bias=c_b[blk + "b"][:, :]
                    )
                    nc.vector.tensor_tensor(
                        he[:, :n], he[:, :n], tb[:, :n], op=OP.add
                    )
                    nc.vector.tensor_copy(hebf[:, :n], he[:, :n])

                nc.sync.dma_start(out[:, s:e], he[:, :n])

    nc.compile()
    return nc


# ----------------------------------------------------------------- kernel ----


def run_cores(cfg, nc, in_maps, trace=False):
    from concourse.bass_utils import run_bass_kernel_spmd

    return run_bass_kernel_spmd(
        nc,
        in_maps,
        core_ids=list(range(cfg.C)),
        trace=trace,
        trace_cores=list(range(cfg.C)) if trace else None,
    )


def assemble(cfg, results):
    outs = []
    for c in range(cfg.C):
        heT = np.asarray(results[c]["out"], np.float32)  # [HID, NPCpad]
        outs.append(heT.T[: cfg.NPC])
    return np.concatenate(outs, axis=0)


def kernel(**inputs) -> np.ndarray:
    cfg = Cfg()
    sched, in_maps = host_prep(cfg, inputs)
    nc = build_program(cfg, sched)
    res = run_cores(cfg, nc, in_maps, trace=False)
    return assemble(cfg, res.results)


if __name__ == "__main__":
    cfg = Cfg()
    print("windows/core:", cfg.W, "NPCpad:", cfg.NPCpad)


# revision 16
# speedup vs baseline: 1.3331x; 1.3331x over previous
"""Bond2AtomLayer GNN message-passing kernel for 8 TRN2 NeuronCores.

Strategy (dst-range sharding, no collectives):
  - Host sorts edges by dst and buckets them to the core that owns the dst
    node range (6250 nodes per core).  Within a core, edges are grouped
    into 128-node destination "windows"; the scatter-sum becomes one-hot
    matmuls accumulating in PSUM per window.
  - Each core computes the k-projection table for ALL nodes (src is
    unsharded) and the q-projection table for its own range, stores them
    in HBM, and per-edge k[src]/q[dst] rows are fetched with gpsimd
    dma_gather (int16 indices; k is split into src<32768 / >=32768
    regions per window since the index type is int16).
  - Edge softmax skips the segment_max pass: z is bounded (|z| < ~10)
    for this model family so exp(z) is safe in fp32, and
    ft = (sum_e bond_e * ez_e) / (sum_e ez_e) per node.
  - The dense residual MLP runs node-parallel in [feat, node] layout.

SPMD: one instruction stream for all 8 cores; the per-window tile counts
in the schedule are the max across cores (host pads with dummy edges).
"""

import math
import numpy as np
import ml_dtypes

BF16 = ml_dtypes.bfloat16

# ---------------------------------------------------------------- config ----


class Cfg:
    def __init__(self, N=50000, E=640000, HID=128, H=8, C=8, S=2, CHUNK=2048,
                 SPLIT=32768):
        self.N, self.E, self.HID, self.H, self.C, self.S = N, E, HID, H, C, S
        self.SPLIT = SPLIT  # int16 gather-index limit
        self.D = HID // H
        assert N % C == 0
        self.NPC = N // C                      # nodes per core
        self.W = math.ceil(self.NPC / 128)     # windows per core
        self.NPCpad = self.W * 128
        self.CHUNK = CHUNK                     # table-build chunk (nodes)
        self.NCH = math.ceil(N / CHUNK)        # k-table chunks
        self.NTAB = self.NCH * CHUNK           # padded k-table rows


class Sched:
    """Common (SPMD) tile schedule: windows grouped S at a time; each
    group's tile order is [lows of each window][highs of each window]."""

    def __init__(self, cfg, low_cnt, high_cnt):
        W, S = cfg.W, cfg.S
        Lw = np.ceil(low_cnt.max(axis=0) / 128.0).astype(np.int64)
        Hw = np.ceil(high_cnt.max(axis=0) / 128.0).astype(np.int64)
        empty = (Lw + Hw) == 0
        Lw[empty] = 1
        self.Lw, self.Hw = Lw, Hw
        self.ngroups = math.ceil(W / S)
        low_base = np.zeros(W, np.int64)
        high_base = np.zeros(W, np.int64)
        self.grp = []  # (t0, tmid, t1)
        t = 0
        for gi in range(self.ngroups):
            ws = range(gi * S, min(W, (gi + 1) * S))
            t0 = t
            for w in ws:
                low_base[w] = t
                t += Lw[w]
            tm = t
            for w in ws:
                high_base[w] = t
                t += Hw[w]
            self.grp.append((t0, tm, t))
        self.T = t
        self.low_base, self.high_base = low_base, high_base
        self.start_tile = np.where(Lw > 0, low_base, high_base)
        self.stop_tile = np.where(Hw > 0, high_base + Hw - 1, low_base + Lw - 1)
        win_of = np.full(self.T, -1, np.int64)
        for w in range(W):
            win_of[low_base[w] : low_base[w] + Lw[w]] = w
            win_of[high_base[w] : high_base[w] + Hw[w]] = w
        self.win_of = win_of
        self.MAXG = max(t1 - t0 for (t0, _, t1) in self.grp)


def _wrap16(arr):
    """int16 index list [L] (L % 16 == 0) -> dma_gather wrapped [128, L/16]."""
    w16 = arr.reshape(-1, 16).T  # [16, L/16]
    return np.tile(w16, (8, 1))


# ------------------------------------------------------------- host prep ----


def host_prep(cfg, inputs):
    N, E, HID, C, W = cfg.N, cfg.E, cfg.HID, cfg.C, cfg.W
    NPC, NPCpad = cfg.NPC, cfg.NPCpad

    src = np.asarray(inputs["src"], np.int32)
    dst = np.asarray(inputs["dst"], np.int32)
    bond = np.asarray(inputs["bond_embedding"], np.float32)
    atom = np.asarray(inputs["atom_embedding"], np.float32)
    dd = np.asarray(inputs["dist_decay"], np.float32).reshape(-1)

    hi = (src >= cfg.SPLIT).astype(np.int64)
    core_of = dst // NPC
    w_of = (dst % NPC) // 128
    key = ((core_of * W + w_of) * 2 + hi)
    order = np.argsort(key, kind="stable")
    s_key = key[order]
    cnt = np.bincount(s_key, minlength=C * W * 2).reshape(C, W, 2)

    sched = Sched(cfg, cnt[:, :, 0], cnt[:, :, 1])
    T = sched.T

    # per-edge slot in the packed [T*128] per-core edge list
    grp_start = np.searchsorted(s_key, np.arange(C * W * 2))
    rank = np.arange(E, dtype=np.int64) - grp_start[s_key]
    s_w = (dst[order] % NPC) // 128
    s_hi = hi[order]
    base_tile = np.where(s_hi == 0, sched.low_base[s_w], sched.high_base[s_w])
    slot = (base_tile + rank // 128) * 128 + rank % 128

    def bf(x):
        return np.ascontiguousarray(np.asarray(x, np.float32)).astype(BF16)

    atomT = np.zeros((HID, cfg.NTAB), BF16)
    atomT[:, :N] = atom.T.astype(BF16)

    shared = {
        "atomT_bf": atomT,
        "k_wT": bf(np.asarray(inputs["k_w"], np.float32).T),
        "q_wT": bf(np.asarray(inputs["q_w"], np.float32).T),
        "k_b_row": bf(np.asarray(inputs["k_b"], np.float32).reshape(1, HID)),
        "q_b_row": bf(np.asarray(inputs["q_b"], np.float32).reshape(1, HID)),
        "attn_gang": bf(
            np.tile(
                np.asarray(inputs["attn"], np.float32).reshape(1, HID),
                (128, sched.MAXG),
            )
        ),
        "iota_row": bf(np.tile(np.arange(128, dtype=np.float32), (128, 1))),
        "identity_bf": bf(np.eye(128, dtype=np.float32)),
        "ones1": bf(np.ones((1, 128), np.float32)),
    }
    for nm in ("lin1", "lin2", "r1a", "r1b", "r2a", "r2b"):
        shared[nm + "_wT"] = bf(np.asarray(inputs[nm + "_w"], np.float32).T)
        shared[nm + "_b"] = np.ascontiguousarray(
            np.asarray(inputs[nm + "_b"], np.float32).reshape(HID, 1)
        )

    s_src = src[order]
    s_dst = dst[order]
    s_dd = dd[order]

    in_maps = []
    for c in range(C):
        m = core_of[order] == c
        sl = slot[m]
        kidx = np.zeros(T * 128, np.int32)
        qidx = np.zeros(T * 128, np.int32)
        dstrel = np.full(T * 128, -1.0, np.float32)
        ddv = np.zeros(T * 128, np.float32)
        bondv = np.zeros((T * 128, HID), np.float32)
        kidx[sl] = np.where(s_src[m] >= cfg.SPLIT, s_src[m] - cfg.SPLIT, s_src[m])
        qidx[sl] = s_dst[m] - c * NPC
        dstrel[sl] = (s_dst[m] - c * NPC - s_w[m] * 128).astype(np.float32)
        ddv[sl] = s_dd[m]
        bondv[sl] = bond[order[m]]

        a0, a1 = c * NPC, min((c + 1) * NPC, N)
        atown = np.zeros((HID, NPCpad), np.float32)
        atown[:, : a1 - a0] = atom[a0:a1].T

        d = {
            "kidx": _wrap16(kidx.astype(np.int16)),
            "qidx": _wrap16(qidx.astype(np.int16)),
            # [128, T] column layouts: tile t, lane p -> [p, t]
            "dstrel": np.ascontiguousarray(dstrel.reshape(T, 128).T),
            "ddv": np.ascontiguousarray(ddv.reshape(T, 128).T),
            # [128, T, HID]: partition-contiguous group loads
            "bondv": np.ascontiguousarray(
                bondv.reshape(T, 128, HID).transpose(1, 0, 2)
            ).astype(BF16),
            "atomT_own_bf": atown.astype(BF16),
            "atomT_own_f32": atown,
        }
        d.update(shared)
        in_maps.append(d)

    return sched, in_maps


# ---------------------------------------------------------------- builder ----


def build_program(cfg, sched):
    import concourse.bass as bass
    import concourse.bacc as bacc
    import concourse.mybir as mybir
    from concourse import library_config
    from concourse.tile import TileContext

    f32 = mybir.dt.float32
    bf16 = mybir.dt.bfloat16
    i16 = mybir.dt.int16
    AF = mybir.ActivationFunctionType
    OP = mybir.AluOpType
    AX = mybir.AxisListType

    HID, H, W, S = cfg.HID, cfg.H, cfg.W, cfg.S
    SPLIT = cfg.SPLIT
    NPCpad, NTAB, NCH, CHUNK = cfg.NPCpad, cfg.NTAB, cfg.NCH, cfg.CHUNK
    T, MAXG = sched.T, sched.MAXG

    nc = bacc.Bacc("TRN2", target_bir_lowering=False, num_swdge_queues=4)

    def inp(name, shape, dt):
        return nc.dram_tensor(name, shape, dt, kind="ExternalInput")

    atomT_bf = inp("atomT_bf", [HID, NTAB], bf16)
    atomT_own_bf = inp("atomT_own_bf", [HID, NPCpad], bf16)
    atomT_own_f32 = inp("atomT_own_f32", [HID, NPCpad], f32)
    kidx = inp("kidx", [128, T * 8], i16)
    qidx = inp("qidx", [128, T * 8], i16)
    dstrel = inp("dstrel", [128, T], f32)
    ddv = inp("ddv", [128, T], f32)
    bondv = inp("bondv", [128, T, HID], bf16)
    k_wT = inp("k_wT", [HID, HID], bf16)
    q_wT = inp("q_wT", [HID, HID], bf16)
    k_b_row = inp("k_b_row", [1, HID], bf16)
    q_b_row = inp("q_b_row", [1, HID], bf16)
    attn_gang = inp("attn_gang", [128, MAXG * HID], bf16)
    iota_row = inp("iota_row", [128, 128], bf16)
    identity_bf = inp("identity_bf", [128, 128], bf16)
    ones1 = inp("ones1", [1, 128], bf16)
    mlp_w = {}
    mlp_b = {}
    for nm in ("lin1", "lin2", "r1a", "r1b", "r2a", "r2b"):
        mlp_w[nm] = inp(nm + "_wT", [HID, HID], bf16)
        mlp_b[nm] = inp(nm + "_b", [HID, 1], f32)

    k_table = nc.dram_tensor("k_table", [NTAB, HID], bf16, kind="Internal")
    q_table = nc.dram_tensor("q_table", [NPCpad, HID], bf16, kind="Internal")
    out = nc.dram_tensor("out", [HID, NPCpad], f32, kind="ExternalOutput")

    with TileContext(nc) as tc:
        with (
            tc.tile_pool(name="const", bufs=1) as cpool,
            tc.tile_pool(name="resident", bufs=1) as rpool,
            tc.tile_pool(name="work", bufs=3) as sb,
            tc.tile_pool(name="edge", bufs=2) as eb,
            tc.tile_pool(name="ps", bufs=2, space="PSUM") as ps,
        ):
            nc.gpsimd.load_library(library_config.mlp)

            c_kwT = cpool.tile_from(k_wT[:, :])
            c_qwT = cpool.tile_from(q_wT[:, :])
            c_kb = cpool.tile_from(k_b_row[:, :])
            c_qb = cpool.tile_from(q_b_row[:, :])
            c_attn = cpool.tile_from(attn_gang[:, :])
            c_iota = cpool.tile_from(iota_row[:, :])
            c_ident = cpool.tile_from(identity_bf[:, :])
            c_ones1 = cpool.tile_from(ones1[:, :])
            c_w = {
                nm: cpool.tile_from(mlp_w[nm][:, :], name="c_w_" + nm)
                for nm in mlp_w
            }
            c_b = {
                nm: cpool.tile_from(mlp_b[nm][:, :], name="c_b_" + nm)
                for nm in mlp_b
            }
            r_kidx = rpool.tile_from(kidx[:, :])
            r_qidx = rpool.tile_from(qidx[:, :])
            r_dstrel = rpool.tile_from(dstrel[:, :])
            r_ddv = rpool.tile_from(ddv[:, :])
            r_ftT = rpool.tile([HID, NPCpad], bf16)

            # ---- phase A/B: projection tables --------------------------
            def build_table(table, srcT, rows, wT, brow):
                nchunks = math.ceil(rows / CHUNK)
                for ch in range(nchunks):
                    cs = ch * CHUNK
                    ce = min(rows, cs + CHUNK)
                    at = sb.tile([128, CHUNK], bf16, tag="atch")
                    nc.sync.dma_start(at[:, : ce - cs], srcT[:, cs:ce])
                    for ti in range(math.ceil((ce - cs) / 128)):
                        pt = ps.tile([128, HID], f32, tag="projp")
                        nc.tensor.matmul(
                            pt[:, :],
                            lhsT=at[:, ti * 128 : (ti + 1) * 128],
                            rhs=wT[:, :],
                            start=True,
                            stop=False,
                        )
                        nc.tensor.matmul(
                            pt[:, :], lhsT=c_ones1[:, :], rhs=brow[:, :],
                            start=False, stop=True,
                        )
                        st = sb.tile([128, HID], bf16, tag="kst")
                        nc.any.tensor_copy(st[:, :], pt[:, :])
                        nc.sync.dma_start(
                            table[cs + ti * 128 : cs + (ti + 1) * 128, :],
                            st[:, :],
                        )

            build_table(k_table, atomT_bf, NTAB, c_kwT, c_kb)
            build_table(q_table, atomT_own_bf, NPCpad, c_qwT, c_qb)

            # ---- phase C: edge pass ------------------------------------
            wp_of = {}
            qrot = [0]
            def next_q():
                qrot[0] = (qrot[0] + 1) % 4
                return qrot[0]
            for gi in range(sched.ngroups):
                t0, tm, t1 = sched.grp[gi]
                nt = t1 - t0
                nlow = tm - t0
                nhigh = t1 - tm
                kg = eb.tile([128, MAXG, HID], bf16, tag="kg")
                if nlow:
                    nc.gpsimd.dma_gather(
                        kg[:, 0:nlow, :],
                        k_table[0:SPLIT, :],
                        r_kidx[:, t0 * 8 : tm * 8],
                        nlow * 128,
                        nlow * 128,
                        HID,
                        single_packet=False,
                        queue_num=next_q(),
                    )
                if nhigh:
                    nc.gpsimd.dma_gather(
                        kg[:, nlow:nt, :],
                        k_table[SPLIT:NTAB, :],
                        r_kidx[:, tm * 8 : t1 * 8],
                        nhigh * 128,
                        nhigh * 128,
                        HID,
                        single_packet=False,
                        queue_num=next_q(),
                    )
                qg = eb.tile([128, MAXG, HID], bf16, tag="qg")
                nc.gpsimd.dma_gather(
                    qg[:, 0:nt, :],
                    q_table[:, :],
                    r_qidx[:, t0 * 8 : t1 * 8],
                    nt * 128,
                    nt * 128,
                    HID,
                    single_packet=False,
                    queue_num=next_q(),
                )
                bo = eb.tile([128, MAXG, HID], bf16, tag="bo")
                nc.sync.dma_start(bo[:, 0:nt, :], bondv[:, t0:t1, :])

                nf = nt * HID
                esum = eb.tile([128, MAXG * HID], bf16, tag="ew")
                nc.vector.tensor_tensor(
                    esum[:, :nf],
                    kg[:, 0:nt, :].rearrange("p t f -> p (t f)"),
                    qg[:, 0:nt, :].rearrange("p t f -> p (t f)"),
                    op=OP.add,
                )
                eact = eb.tile([128, MAXG * HID], bf16, tag="eact")
                nc.scalar.activation(
                    eact[:, :nf], esum[:, :nf], AF.Lrelu, alpha=0.01
                )
                ew = eb.tile([128, MAXG * HID], bf16, tag="ew")
                nc.vector.tensor_tensor(
                    ew[:, :nf], eact[:, :nf], c_attn[:, :nf], op=OP.mult
                )
                att = eb.tile([128, MAXG, H], f32, tag="att")
                nc.vector.reduce_sum(
                    att[:, 0:nt, :],
                    ew[:, :nf].rearrange("p (a d) -> p a d", d=cfg.D),
                    axis=AX.X,
                )
                z = eb.tile([128, MAXG, H], f32, tag="z")
                nc.vector.tensor_tensor(
                    z[:, 0:nt, :],
                    att[:, 0:nt, :],
                    r_ddv[:, t0:t1].broadcast_to([128, nt, H]),
                    op=OP.add,
                )
                ez = eb.tile([128, MAXG, H], bf16, tag="ez")
                nc.scalar.activation(ez[:, 0:nt, :], z[:, 0:nt, :], AF.Exp)
                # m = bond * ez (broadcast over D), in place on bo (gpsimd)
                nc.gpsimd.tensor_tensor(
                    bo[:, 0:nt, :].rearrange("p t (h d) -> p t h d", h=H),
                    bo[:, 0:nt, :].rearrange("p t (h d) -> p t h d", h=H),
                    ez[:, 0:nt, :].broadcast_to([128, nt, H, cfg.D]),
                    op=OP.mult,
                )
                for b in range(nt):
                    t = t0 + b
                    w = int(sched.win_of[t])
                    first = t == int(sched.start_tile[w])
                    last = t == int(sched.stop_tile[w])
                    oh = eb.tile([128, 128], bf16, tag="oh", bufs=6)
                    nc.vector.tensor_scalar(
                        oh[:, :],
                        c_iota[:, :],
                        r_dstrel[:, t : t + 1],
                        None,
                        op0=OP.is_equal,
                    )
                    if first:
                        wp_of[w] = ps.tile(
                            [128, HID + H], f32, tag="wp", bufs=4, name="wp"
                        )
                    wp = wp_of[w]
                    # one start/stop per PSUM bank: start=True lazily zeroes
                    # the whole 2KB zero-region, so only the first (num)
                    # matmul starts and only the final (den) matmul stops.
                    nc.tensor.matmul(
                        wp[:, 0:HID], lhsT=oh[:, :], rhs=bo[:, b, :],
                        start=first, stop=False,
                    )
                    nc.tensor.matmul(
                        wp[:, HID : HID + H], lhsT=oh[:, :], rhs=ez[:, b, :],
                        start=False, stop=last,
                    )
                    if last:
                        den = sb.tile([128, H], f32, tag="den")
                        nc.vector.tensor_scalar(
                            den[:, :], wp[:, HID : HID + H], 1e-6, None,
                            op0=OP.add,
                        )
                        recip = sb.tile([128, H], f32, tag="recip")
                        nc.vector.reciprocal(recip[:, :], den[:, :])
                        ftw = sb.tile([128, HID], bf16, tag="ftw")
                        nc.vector.tensor_tensor(
                            ftw[:, :].rearrange("p (h d) -> p h d", h=H),
                            wp[:, 0:HID].rearrange("p (h d) -> p h d", h=H),
                            recip[:, :].broadcast_to([128, H, cfg.D]),
                            op=OP.mult,
                        )
                        ftp = ps.tile([128, 128], bf16, tag="ftp", bufs=1)
                        nc.tensor.transpose(
                            ftp[:, :], ftw[:, :], identity=c_ident[:, :]
                        )
                        nc.any.tensor_copy(
                            r_ftT[:, w * 128 : (w + 1) * 128], ftp[:, :]
                        )
                        del wp_of[w]

            # ---- phase D: dense output path + residual MLPs ------------
            NCHK = 512
            nchunks = math.ceil(NPCpad / NCHK)
            for ci in range(nchunks):
                s = ci * NCHK
                e = min(NPCpad, s + NCHK)
                n = e - s
                p1 = ps.tile([128, NCHK], f32, tag="projp")
                nc.tensor.matmul(
                    p1[:, :n], lhsT=c_w["lin1"][:, :], rhs=r_ftT[:, s:e],
                    start=True, stop=True,
                )
                t1 = sb.tile([128, NCHK], bf16, tag="t1")
                nc.scalar.activation(
                    t1[:, :n], p1[:, :n], AF.Relu, bias=c_b["lin1"][:, :]
                )
                p2 = ps.tile([128, NCHK], f32, tag="projq", bufs=1)
                nc.tensor.matmul(
                    p2[:, :n], lhsT=c_w["lin2"][:, :], rhs=t1[:, :n],
                    start=True, stop=True,
                )
                h2 = sb.tile([128, NCHK], f32, tag="h2")
                nc.scalar.activation(
                    h2[:, :n], p2[:, :n], AF.Identity, bias=c_b["lin2"][:, :]
                )
                atf = sb.tile([128, NCHK], f32, tag="atf")
                nc.sync.dma_start(atf[:, :n], atomT_own_f32[:, s:e])
                he = sb.tile([128, NCHK], f32, tag="he")
                nc.vector.tensor_tensor(
                    he[:, :n], h2[:, :n], atf[:, :n], op=OP.add
                )
                hebf = sb.tile([128, NCHK], bf16, tag="hebf")
                nc.vector.tensor_copy(hebf[:, :n], he[:, :n])

                for blk in ("r1", "r2"):
                    pa = ps.tile([128, NCHK], f32, tag="projp")
                    nc.tensor.matmul(
                        pa[:, :n], lhsT=c_w[blk + "a"][:, :], rhs=hebf[:, :n],
                        start=True, stop=True,
                    )
                    ta = sb.tile([128, NCHK], bf16, tag="t1")
                    nc.scalar.activation(
                        ta[:, :n], pa[:, :n], AF.Relu, bias=c_b[blk + "a"][:, :]
                    )
                    pb = ps.tile([128, NCHK], f32, tag="projq", bufs=1)
                    nc.tensor.matmul(
                        pb[:, :n], lhsT=c_w[blk + "b"][:, :], rhs=ta[:, :n],
                        start=True, stop=True,
                    )
                    tb = sb.tile([128, NCHK], bf16, tag="t2")
                    nc.scalar.activation(
                        tb[:, :n], pb[:, :n], AF.Relu, bias=c_b[blk + "b"][:, :]
                    )
                    nc.vector.tensor_tensor(
                        he[:, :n], he[:, :n], tb[:, :n], op=OP.add
                    )
                    nc.vector.tensor_copy(hebf[:, :n], he[:, :n])

                nc.sync.dma_start(out[:, s:e], he[:, :n])

    nc.compile()
    return nc


# ----------------------------------------------------------------- kernel ----


def run_cores(cfg, nc, in_maps, trace=False):
    from concourse.bass_utils import run_bass_kernel_spmd

    return run_bass_kernel_spmd(
        nc,
        in_maps,
        core_ids=list(range(cfg.C)),
        trace=trace,
        trace_cores=list(range(cfg.C)) if trace else None,
    )


def assemble(cfg, results):
    outs = []
    for c in range(cfg.C):
        heT = np.asarray(results[c]["out"], np.float32)  # [HID, NPCpad]
        outs.append(heT.T[: cfg.NPC])
    return np.concatenate(outs, axis=0)


def kernel(**inputs) -> np.ndarray:
    cfg = Cfg()
    sched, in_maps = host_prep(cfg, inputs)
    nc = build_program(cfg, sched)
    res = run_cores(cfg, nc, in_maps, trace=False)
    return assemble(cfg, res.results)


if __name__ == "__main__":
    cfg = Cfg()
    print("windows/core:", cfg.W, "NPCpad:", cfg.NPCpad)


# revision 18
# speedup vs baseline: 1.6290x; 1.2220x over previous
"""Bond2AtomLayer GNN message-passing kernel for 8 TRN2 NeuronCores.

Strategy (dst-range sharding, no collectives):
  - Host sorts edges by dst and buckets them to the core that owns the dst
    node range (6250 nodes per core).  Within a core, edges are grouped
    into 128-node destination "windows"; the scatter-sum becomes one-hot
    matmuls accumulating in PSUM per window.
  - Each core computes the k-projection table for ALL nodes (src is
    unsharded) and the q-projection table for its own range, stores them
    in HBM, and per-edge k[src]/q[dst] rows are fetched with gpsimd
    dma_gather (int16 indices; k is split into src<32768 / >=32768
    regions per window since the index type is int16).
  - Edge softmax skips the segment_max pass: z is bounded (|z| < ~10)
    for this model family so exp(z) is safe in fp32, and
    ft = (sum_e bond_e * ez_e) / (sum_e ez_e) per node.
  - The dense residual MLP runs node-parallel in [feat, node] layout.

SPMD: one instruction stream for all 8 cores; the per-window tile counts
in the schedule are the max across cores (host pads with dummy edges).
"""

import math
import numpy as np
import ml_dtypes

BF16 = ml_dtypes.bfloat16

# ---------------------------------------------------------------- config ----


class Cfg:
    def __init__(self, N=50000, E=640000, HID=128, H=8, C=8, S=2, CHUNK=2048,
                 SPLIT=32768):
        self.N, self.E, self.HID, self.H, self.C, self.S = N, E, HID, H, C, S
        self.SPLIT = SPLIT  # int16 gather-index limit
        self.D = HID // H
        assert N % C == 0
        self.NPC = N // C                      # nodes per core
        self.W = math.ceil(self.NPC / 128)     # windows per core
        self.NPCpad = self.W * 128
        self.CHUNK = CHUNK                     # table-build chunk (nodes)
        self.NCH = math.ceil(N / CHUNK)        # k-table chunks
        self.NTAB = self.NCH * CHUNK           # padded k-table rows


class Sched:
    """Common (SPMD) tile schedule: windows grouped S at a time; each
    group's tile order is [lows of each window][highs of each window]."""

    def __init__(self, cfg, low_cnt, high_cnt):
        W, S = cfg.W, cfg.S
        Lw = np.ceil(low_cnt.max(axis=0) / 128.0).astype(np.int64)
        Hw = np.ceil(high_cnt.max(axis=0) / 128.0).astype(np.int64)
        empty = (Lw + Hw) == 0
        Lw[empty] = 1
        self.Lw, self.Hw = Lw, Hw
        self.ngroups = math.ceil(W / S)
        low_base = np.zeros(W, np.int64)
        high_base = np.zeros(W, np.int64)
        self.grp = []  # (t0, tmid, t1)
        t = 0
        for gi in range(self.ngroups):
            ws = range(gi * S, min(W, (gi + 1) * S))
            t0 = t
            for w in ws:
                low_base[w] = t
                t += Lw[w]
            tm = t
            for w in ws:
                high_base[w] = t
                t += Hw[w]
            self.grp.append((t0, tm, t))
        self.T = t
        self.low_base, self.high_base = low_base, high_base
        self.start_tile = np.where(Lw > 0, low_base, high_base)
        self.stop_tile = np.where(Hw > 0, high_base + Hw - 1, low_base + Lw - 1)
        win_of = np.full(self.T, -1, np.int64)
        for w in range(W):
            win_of[low_base[w] : low_base[w] + Lw[w]] = w
            win_of[high_base[w] : high_base[w] + Hw[w]] = w
        self.win_of = win_of
        self.MAXG = max(t1 - t0 for (t0, _, t1) in self.grp)


def _wrap16(arr):
    """int16 index list [L] (L % 16 == 0) -> dma_gather wrapped [128, L/16]."""
    w16 = arr.reshape(-1, 16).T  # [16, L/16]
    return np.tile(w16, (8, 1))


# ------------------------------------------------------------- host prep ----


def host_prep(cfg, inputs):
    N, E, HID, C, W = cfg.N, cfg.E, cfg.HID, cfg.C, cfg.W
    NPC, NPCpad = cfg.NPC, cfg.NPCpad

    src = np.asarray(inputs["src"], np.int32)
    dst = np.asarray(inputs["dst"], np.int32)
    bond = np.asarray(inputs["bond_embedding"], np.float32)
    atom = np.asarray(inputs["atom_embedding"], np.float32)
    dd = np.asarray(inputs["dist_decay"], np.float32).reshape(-1)

    hi = (src >= cfg.SPLIT).astype(np.int64)
    core_of = dst // NPC
    w_of = (dst % NPC) // 128
    key = ((core_of * W + w_of) * 2 + hi)
    order = np.argsort(key, kind="stable")
    s_key = key[order]
    cnt = np.bincount(s_key, minlength=C * W * 2).reshape(C, W, 2)

    sched = Sched(cfg, cnt[:, :, 0], cnt[:, :, 1])
    T = sched.T

    # per-edge slot in the packed [T*128] per-core edge list
    grp_start = np.searchsorted(s_key, np.arange(C * W * 2))
    rank = np.arange(E, dtype=np.int64) - grp_start[s_key]
    s_w = (dst[order] % NPC) // 128
    s_hi = hi[order]
    base_tile = np.where(s_hi == 0, sched.low_base[s_w], sched.high_base[s_w])
    slot = (base_tile + rank // 128) * 128 + rank % 128

    def bf(x):
        return np.ascontiguousarray(np.asarray(x, np.float32)).astype(BF16)

    atomT = np.zeros((HID, cfg.NTAB), BF16)
    atomT[:, :N] = atom.T.astype(BF16)

    shared = {
        "atomT_bf": atomT,
        "k_wT": bf(np.asarray(inputs["k_w"], np.float32).T),
        "q_wT": bf(np.asarray(inputs["q_w"], np.float32).T),
        "k_b_row": bf(np.asarray(inputs["k_b"], np.float32).reshape(1, HID)),
        "q_b_row": bf(np.asarray(inputs["q_b"], np.float32).reshape(1, HID)),
        "attn_gang": bf(
            np.tile(
                np.asarray(inputs["attn"], np.float32).reshape(1, HID),
                (128, sched.MAXG),
            )
        ),
        "iota_row": bf(np.tile(np.arange(128, dtype=np.float32), (128, 1))),
        "identity_bf": bf(np.eye(128, dtype=np.float32)),
        "ones1": bf(np.ones((1, 128), np.float32)),
    }
    for nm in ("lin1", "lin2", "r1a", "r1b", "r2a", "r2b"):
        shared[nm + "_wT"] = bf(np.asarray(inputs[nm + "_w"], np.float32).T)
        shared[nm + "_b"] = np.ascontiguousarray(
            np.asarray(inputs[nm + "_b"], np.float32).reshape(HID, 1)
        )

    s_src = src[order]
    s_dst = dst[order]
    s_dd = dd[order]

    in_maps = []
    for c in range(C):
        m = core_of[order] == c
        sl = slot[m]
        kidx = np.zeros(T * 128, np.int32)
        qidx = np.zeros(T * 128, np.int32)
        dstrel = np.full(T * 128, -1.0, np.float32)
        ddv = np.zeros(T * 128, np.float32)
        bondv = np.zeros((T * 128, HID), np.float32)
        kidx[sl] = np.where(s_src[m] >= cfg.SPLIT, s_src[m] - cfg.SPLIT, s_src[m])
        qidx[sl] = s_dst[m] - c * NPC
        dstrel[sl] = (s_dst[m] - c * NPC - s_w[m] * 128).astype(np.float32)
        ddv[sl] = s_dd[m]
        bondv[sl] = bond[order[m]]

        a0, a1 = c * NPC, min((c + 1) * NPC, N)
        atown = np.zeros((HID, NPCpad), np.float32)
        atown[:, : a1 - a0] = atom[a0:a1].T

        d = {
            "kidx": _wrap16(kidx.astype(np.int16)),
            "qidx": _wrap16(qidx.astype(np.int16)),
            # [128, T] column layouts: tile t, lane p -> [p, t]
            "dstrel": np.ascontiguousarray(dstrel.reshape(T, 128).T),
            "ddv": np.ascontiguousarray(ddv.reshape(T, 128).T),
            # [128, T, HID]: partition-contiguous group loads
            "bondv": np.ascontiguousarray(
                bondv.reshape(T, 128, HID).transpose(1, 0, 2)
            ).astype(BF16),
            "atomT_own_bf": atown.astype(BF16),
            "atomT_own_f32": atown,
        }
        d.update(shared)
        in_maps.append(d)

    return sched, in_maps


# ---------------------------------------------------------------- builder ----


def build_program(cfg, sched):
    import concourse.bass as bass
    import concourse.bacc as bacc
    import concourse.mybir as mybir
    from concourse import library_config
    from concourse.tile import TileContext

    f32 = mybir.dt.float32
    bf16 = mybir.dt.bfloat16
    i16 = mybir.dt.int16
    AF = mybir.ActivationFunctionType
    OP = mybir.AluOpType
    AX = mybir.AxisListType

    HID, H, W, S = cfg.HID, cfg.H, cfg.W, cfg.S
    SPLIT = cfg.SPLIT
    NPCpad, NTAB, NCH, CHUNK = cfg.NPCpad, cfg.NTAB, cfg.NCH, cfg.CHUNK
    T, MAXG = sched.T, sched.MAXG

    nc = bacc.Bacc("TRN2", target_bir_lowering=False, num_swdge_queues=4)

    def inp(name, shape, dt):
        return nc.dram_tensor(name, shape, dt, kind="ExternalInput")

    atomT_bf = inp("atomT_bf", [HID, NTAB], bf16)
    atomT_own_bf = inp("atomT_own_bf", [HID, NPCpad], bf16)
    atomT_own_f32 = inp("atomT_own_f32", [HID, NPCpad], f32)
    kidx = inp("kidx", [128, T * 8], i16)
    qidx = inp("qidx", [128, T * 8], i16)
    dstrel = inp("dstrel", [128, T], f32)
    ddv = inp("ddv", [128, T], f32)
    bondv = inp("bondv", [128, T, HID], bf16)
    k_wT = inp("k_wT", [HID, HID], bf16)
    q_wT = inp("q_wT", [HID, HID], bf16)
    k_b_row = inp("k_b_row", [1, HID], bf16)
    q_b_row = inp("q_b_row", [1, HID], bf16)
    attn_gang = inp("attn_gang", [128, MAXG * HID], bf16)
    iota_row = inp("iota_row", [128, 128], bf16)
    identity_bf = inp("identity_bf", [128, 128], bf16)
    ones1 = inp("ones1", [1, 128], bf16)
    mlp_w = {}
    mlp_b = {}
    for nm in ("lin1", "lin2", "r1a", "r1b", "r2a", "r2b"):
        mlp_w[nm] = inp(nm + "_wT", [HID, HID], bf16)
        mlp_b[nm] = inp(nm + "_b", [HID, 1], f32)

    k_table = nc.dram_tensor("k_table", [NTAB, HID], bf16, kind="Internal")
    q_table = nc.dram_tensor("q_table", [NPCpad, HID], bf16, kind="Internal")
    out = nc.dram_tensor("out", [HID, NPCpad], f32, kind="ExternalOutput")

    with TileContext(nc) as tc:
        with (
            tc.tile_pool(name="const", bufs=1) as cpool,
            tc.tile_pool(name="resident", bufs=1) as rpool,
            tc.tile_pool(name="work", bufs=3) as sb,
            tc.tile_pool(name="edge", bufs=2) as eb,
            tc.tile_pool(name="ps", bufs=2, space="PSUM") as ps,
        ):
            nc.gpsimd.load_library(library_config.mlp)

            c_kwT = cpool.tile_from(k_wT[:, :])
            c_qwT = cpool.tile_from(q_wT[:, :])
            c_kb = cpool.tile_from(k_b_row[:, :])
            c_qb = cpool.tile_from(q_b_row[:, :])
            c_attn = cpool.tile_from(attn_gang[:, :])
            c_iota = cpool.tile_from(iota_row[:, :])
            c_ident = cpool.tile_from(identity_bf[:, :])
            c_ones1 = cpool.tile_from(ones1[:, :])
            c_w = {
                nm: cpool.tile_from(mlp_w[nm][:, :], name="c_w_" + nm)
                for nm in mlp_w
            }
            c_b = {
                nm: cpool.tile_from(mlp_b[nm][:, :], name="c_b_" + nm)
                for nm in mlp_b
            }
            r_kidx = rpool.tile_from(kidx[:, :])
            r_qidx = rpool.tile_from(qidx[:, :])
            r_dstrel = rpool.tile_from(dstrel[:, :])
            r_ddv = rpool.tile_from(ddv[:, :])
            r_ftT = rpool.tile([HID, NPCpad], bf16)

            # ---- phase A/B: projection tables --------------------------
            def build_table(table, srcT, rows, wT, brow):
                nchunks = math.ceil(rows / CHUNK)
                for ch in range(nchunks):
                    cs = ch * CHUNK
                    ce = min(rows, cs + CHUNK)
                    at = sb.tile([128, CHUNK], bf16, tag="atch")
                    nc.sync.dma_start(at[:, : ce - cs], srcT[:, cs:ce])
                    for ti in range(math.ceil((ce - cs) / 128)):
                        pt = ps.tile([128, HID], f32, tag="projp")
                        nc.tensor.matmul(
                            pt[:, :],
                            lhsT=at[:, ti * 128 : (ti + 1) * 128],
                            rhs=wT[:, :],
                            start=True,
                            stop=False,
                        )
                        nc.tensor.matmul(
                            pt[:, :], lhsT=c_ones1[:, :], rhs=brow[:, :],
                            start=False, stop=True,
                        )
                        st = sb.tile([128, HID], bf16, tag="kst")
                        if ti % 2 == 0:
                            nc.scalar.copy(st[:, :], pt[:, :])
                        else:
                            nc.vector.tensor_copy(st[:, :], pt[:, :])
                        nc.sync.dma_start(
                            table[cs + ti * 128 : cs + (ti + 1) * 128, :],
                            st[:, :],
                        )

            build_table(k_table, atomT_bf, NTAB, c_kwT, c_kb)
            build_table(q_table, atomT_own_bf, NPCpad, c_qwT, c_qb)

            # ---- phase C: edge pass ------------------------------------
            wp_of = {}
            qrot = [0]
            def next_q():
                qrot[0] = (qrot[0] + 1) % 4
                return qrot[0]
            for gi in range(sched.ngroups):
                t0, tm, t1 = sched.grp[gi]
                nt = t1 - t0
                nlow = tm - t0
                nhigh = t1 - tm
                GCH = 8  # tiles per gather chunk (<=1024 idx)
                kg = eb.tile([128, MAXG, HID], bf16, tag="kg")

                def gather_chunks(dst, dst0, tab_ap, idx, i0, i1):
                    for cs in range(i0, i1, GCH):
                        ce = min(i1, cs + GCH)
                        nc.gpsimd.dma_gather(
                            dst[:, dst0 + cs - i0 : dst0 + ce - i0, :],
                            tab_ap,
                            idx[:, cs * 8 : ce * 8],
                            (ce - cs) * 128,
                            (ce - cs) * 128,
                            HID,
                            single_packet=False,
                            queue_num=next_q(),
                        )

                if nlow:
                    gather_chunks(kg, 0, k_table[0:SPLIT, :], r_kidx, t0, tm)
                if nhigh:
                    gather_chunks(kg, nlow, k_table[SPLIT:NTAB, :], r_kidx, tm, t1)
                qg = eb.tile([128, MAXG, HID], bf16, tag="qg")
                gather_chunks(qg, 0, q_table[:, :], r_qidx, t0, t1)
                bo = eb.tile([128, MAXG, HID], bf16, tag="bo")
                nc.sync.dma_start(bo[:, 0:nt, :], bondv[:, t0:t1, :])

                nf = nt * HID
                esum = eb.tile([128, MAXG * HID], bf16, tag="ew")
                nc.vector.tensor_tensor(
                    esum[:, :nf],
                    kg[:, 0:nt, :].rearrange("p t f -> p (t f)"),
                    qg[:, 0:nt, :].rearrange("p t f -> p (t f)"),
                    op=OP.add,
                )
                eact = eb.tile([128, MAXG * HID], bf16, tag="eact")
                nc.scalar.activation(
                    eact[:, :nf], esum[:, :nf], AF.Lrelu, alpha=0.01
                )
                ew = eb.tile([128, MAXG * HID], bf16, tag="ew")
                nc.vector.tensor_tensor(
                    ew[:, :nf], eact[:, :nf], c_attn[:, :nf], op=OP.mult
                )
                att = eb.tile([128, MAXG, H], f32, tag="att")
                nc.vector.reduce_sum(
                    att[:, 0:nt, :],
                    ew[:, :nf].rearrange("p (a d) -> p a d", d=cfg.D),
                    axis=AX.X,
                )
                z = eb.tile([128, MAXG, H], f32, tag="z")
                nc.vector.tensor_tensor(
                    z[:, 0:nt, :],
                    att[:, 0:nt, :],
                    r_ddv[:, t0:t1].broadcast_to([128, nt, H]),
                    op=OP.add,
                )
                ez = eb.tile([128, MAXG, H], bf16, tag="ez")
                nc.scalar.activation(ez[:, 0:nt, :], z[:, 0:nt, :], AF.Exp)
                # m = bond * ez (broadcast over D), in place on bo (gpsimd)
                nc.gpsimd.tensor_tensor(
                    bo[:, 0:nt, :].rearrange("p t (h d) -> p t h d", h=H),
                    bo[:, 0:nt, :].rearrange("p t (h d) -> p t h d", h=H),
                    ez[:, 0:nt, :].broadcast_to([128, nt, H, cfg.D]),
                    op=OP.mult,
                )
                oh = eb.tile([128, MAXG, 128], bf16, tag="oh")
                nc.vector.tensor_tensor(
                    oh[:, 0:nt, :],
                    c_iota[:, None, :].to_broadcast([128, nt, 128]),
                    r_dstrel[:, t0:t1].broadcast_to([128, nt, 128]),
                    op=OP.is_equal,
                )
                for b in range(nt):
                    t = t0 + b
                    w = int(sched.win_of[t])
                    first = t == int(sched.start_tile[w])
                    last = t == int(sched.stop_tile[w])
                    if first:
                        wp_of[w] = ps.tile(
                            [128, HID + H], f32, tag="wp", bufs=4, name="wp"
                        )
                    wp = wp_of[w]
                    # one start/stop per PSUM bank: start=True lazily zeroes
                    # the whole 2KB zero-region, so only the first (num)
                    # matmul starts and only the final (den) matmul stops.
                    nc.tensor.matmul(
                        wp[:, 0:HID], lhsT=oh[:, b, :], rhs=bo[:, b, :],
                        start=first, stop=False,
                    )
                    nc.tensor.matmul(
                        wp[:, HID : HID + H], lhsT=oh[:, b, :], rhs=ez[:, b, :],
                        start=False, stop=last,
                    )
                    if last:
                        den = sb.tile([128, H], f32, tag="den")
                        nc.vector.tensor_scalar(
                            den[:, :], wp[:, HID : HID + H], 1e-6, None,
                            op0=OP.add,
                        )
                        recip = sb.tile([128, H], f32, tag="recip")
                        nc.vector.reciprocal(recip[:, :], den[:, :])
                        ftw = sb.tile([128, HID], bf16, tag="ftw")
                        nc.vector.tensor_tensor(
                            ftw[:, :].rearrange("p (h d) -> p h d", h=H),
                            wp[:, 0:HID].rearrange("p (h d) -> p h d", h=H),
                            recip[:, :].broadcast_to([128, H, cfg.D]),
                            op=OP.mult,
                        )
                        ftp = ps.tile([128, 128], bf16, tag="ftp", bufs=1)
                        nc.tensor.transpose(
                            ftp[:, :], ftw[:, :], identity=c_ident[:, :]
                        )
                        nc.any.tensor_copy(
                            r_ftT[:, w * 128 : (w + 1) * 128], ftp[:, :]
                        )
                        del wp_of[w]

            # ---- phase D: dense output path + residual MLPs ------------
            NCHK = 512
            nchunks = math.ceil(NPCpad / NCHK)
            for ci in range(nchunks):
                s = ci * NCHK
                e = min(NPCpad, s + NCHK)
                n = e - s
                p1 = ps.tile([128, NCHK], f32, tag="projp")
                nc.tensor.matmul(
                    p1[:, :n], lhsT=c_w["lin1"][:, :], rhs=r_ftT[:, s:e],
                    start=True, stop=True,
                )
                t1 = sb.tile([128, NCHK], bf16, tag="t1")
                nc.scalar.activation(
                    t1[:, :n], p1[:, :n], AF.Relu, bias=c_b["lin1"][:, :]
                )
                p2 = ps.tile([128, NCHK], f32, tag="projq", bufs=1)
                nc.tensor.matmul(
                    p2[:, :n], lhsT=c_w["lin2"][:, :], rhs=t1[:, :n],
                    start=True, stop=True,
                )
                h2 = sb.tile([128, NCHK], f32, tag="h2")
                nc.scalar.activation(
                    h2[:, :n], p2[:, :n], AF.Identity, bias=c_b["lin2"][:, :]
                )
                atf = sb.tile([128, NCHK], f32, tag="atf")
                nc.sync.dma_start(atf[:, :n], atomT_own_f32[:, s:e])
                he = sb.tile([128, NCHK], f32, tag="he")
                nc.vector.tensor_tensor(
                    he[:, :n], h2[:, :n], atf[:, :n], op=OP.add
                )
                hebf = sb.tile([128, NCHK], bf16, tag="hebf")
                nc.vector.tensor_copy(hebf[:, :n], he[:, :n])

                for blk in ("r1", "r2"):
                    pa = ps.tile([128, NCHK], f32, tag="projp")
                    nc.tensor.matmul(
                        pa[:, :n], lhsT=c_w[blk + "a"][:, :], rhs=hebf[:, :n],
                        start=True, stop=True,
                    )
                    ta = sb.tile([128, NCHK], bf16, tag="t1")
                    nc.scalar.activation(
                        ta[:, :n], pa[:, :n], AF.Relu, bias=c_b[blk + "a"][:, :]
                    )
                    pb = ps.tile([128, NCHK], f32, tag="projq", bufs=1)
                    nc.tensor.matmul(
                        pb[:, :n], lhsT=c_w[blk + "b"][:, :], rhs=ta[:, :n],
                        start=True, stop=True,
                    )
                    tb = sb.tile([128, NCHK], bf16, tag="t2")
                    nc.scalar.activation(
                        tb[:, :n], pb[:, :n], AF.Relu, bias=c_b[blk + "b"][:, :]
                    )
                    nc.vector.tensor_tensor(
                        he[:, :n], he[:, :n], tb[:, :n], op=OP.add
                    )
                    nc.vector.tensor_copy(hebf[:, :n], he[:, :n])

                nc.sync.dma_start(out[:, s:e], he[:, :n])

    nc.compile()
    return nc


# ----------------------------------------------------------------- kernel ----


def run_cores(cfg, nc, in_maps, trace=False):
    from concourse.bass_utils import run_bass_kernel_spmd

    return run_bass_kernel_spmd(
        nc,
        in_maps,
        core_ids=list(range(cfg.C)),
        trace=trace,
        trace_cores=list(range(cfg.C)) if trace else None,
    )


def assemble(cfg, results):
    outs = []
    for c in range(cfg.C):
        heT = np.asarray(results[c]["out"], np.float32)  # [HID, NPCpad]
        outs.append(heT.T[: cfg.NPC])
    return np.concatenate(outs, axis=0)


def kernel(**inputs) -> np.ndarray:
    cfg = Cfg()
    sched, in_maps = host_prep(cfg, inputs)
    nc = build_program(cfg, sched)
    res = run_cores(cfg, nc, in_maps, trace=False)
    return assemble(cfg, res.results)


if __name__ == "__main__":
    cfg = Cfg()
    print("windows/core:", cfg.W, "NPCpad:", cfg.NPCpad)
